# revision 6
# baseline (speedup 1.0000x reference)
"""Multi-head attention Trainium2 kernel (8 NeuronCores, Bass/Tile).

Sharding: core c -> (batch b = c//2, head-group hg = c%2). Each core computes
attention for 8 of the 16 heads of one batch element plus its partial
out-projection; the host sums the two head-group partials per batch.

Per-core layouts (host pre-transposes inputs; contraction dims on partitions):
  xT  [E=1024, S=2048]      x[b].T
  wqT/wkT/wvT [1024, 512]   W[hg_rows].T
  woT [512, 1024]           Wo[:, hg_cols].T

On-chip pipeline (bf16 matmul operands, fp32 PSUM accumulation/softmax):
  QT = wqT.T-tiles @ xT   [512, 2048] (head-major, transposed)
  KT likewise; V natural [2048, 512]
  scoresT[t,s] = KT_h.T-tile @ QT_h   (K=64; two heads row-packed -> concurrent)
  expT = exp(scoresT/8) on ScalarE    ([128,1024] psum->sbuf per t-tile)
  outT = V_h.T @ expT                 (col-packed pair -> concurrent)
  Z    = ones.T @ expT                (M=1 col-tiled pair, rides in PE gaps)
  outT_norm = outT * rep(1/Z)         (K=2 selector matmul + DVE mult)
  out = outT_norm.T-tiles @ woT       [2048, 1024] partial, stored bf16

Schedule: 16 single-pair groups (pair p, query-chunk n) in anti-diagonal
order so upfront projections are minimal (K pair0, Q pair0-chunk0, V t0-3,
~13us); every other projection, the per-pair softmax normalization, and the
out-projection drip into the attention steps as fine-grained (<=2 matmul)
filler so ScalarE's exp stream paces the kernel. ensure() force-emits a
group's K/Q chains and each step's V chain before their consumers, keeping
the in-order PE stream deadlock-free. PE p-state is warmed and the Exp
table preloaded during the initial x DMA. Output is written in bf16 (halves
the output DMA); the host accumulates the two head-group partials in fp32.
"""

import os
import sys
import types

import numpy as np

B, S, E, H = 4, 2048, 1024, 16
DK = E // H  # 64
HG = H // 2  # heads per core = 8
DG = HG * DK  # 512 projected dims per core
NP = HG // 2  # head-pairs per core = 4
NCORES = 8

TRACE = bool(os.environ.get("TRN_KERNEL_TRACE"))
# matmul-operand dtype: bf16 single-pass PE (fp32 PSUM accumulation) vs
# fp32 operands (PE double-pumps each matmul -> ~2x slower)
MM_DTYPE = os.environ.get("TRN_MM_DTYPE", "bf16")
LAST_EXEC_TIME_NS = None

_cache = {}

_SEL2 = np.zeros((2, 128), dtype=np.float32)
_SEL2[0, 0:64] = 1.0
_SEL2[1, 64:128] = 1.0


def _env_setup():
    import antenv

    if "antenv.axon_hooks" not in sys.modules:
        mod = types.ModuleType("antenv.axon_hooks")
        mod._hook = None
        mod.set_axon_ntff_profile_hook = lambda h: setattr(mod, "_hook", h)
        mod.get_axon_ntff_profile_hook = lambda: mod._hook
        sys.modules["antenv.axon_hooks"] = mod
        antenv.axon_hooks = mod
        try:
            from trn_agent_boot.trn_boot import _ntff_profile_via_ctypes

            mod.set_axon_ntff_profile_hook(
                _ntff_profile_via_ctypes("/opt/axon/libaxon_pjrt.so")
            )
        except Exception:
            pass

    import concourse.bass_utils as bass_utils

    bass_utils.upload_artifacts = lambda tmpdir: tmpdir

    import concourse.tile as tile
    from concourse import mybir
    from concourse.vector_clock import ScopedClock

    if getattr(tile.TileContext, "_wait_split_patched", False):
        return

    MAX_WAITS = 1  # walrus on this image rejects >1 sync wait per instruction

    def _drain_and_barrier_split(self, tick_clock, wait_clock):
        probe = self.nc.sync.drain()
        wait_clock.add_sem_waits(
            probe.ins, ScopedClock({None: tick_clock.global_clock})
        )
        waits = list(probe.ins.sync_info.on_wait)
        if len(waits) > MAX_WAITS:
            num2h = {h.num: h for h in self.sems.allocated().values()}
            probe.ins.sync_info.on_wait = []
            for w in waits:
                self.nc.sync.wait_ge(num2h[w.id], w.wait_value)
            self.nc.sync.drain()
        self.nc.all_engine_barrier()
        popped = self.nc._tile_sem_poison_stack.pop()
        assert popped is self._sem_poison
        self.nc.clear_and_free_semaphores(list(self.sems.allocated().values()))
        self.nc.all_engine_barrier()

    _orig_commit = tile.TileContext._commit_instruction
    _ctr = [0]

    def _commit_split_waits(self, inst, lazy_reg_writes=True):
        si = inst.sync_info
        if (
            si is not None
            and len(si.on_wait) > MAX_WAITS
            and inst.engine != mybir.EngineType.Unassigned
        ):
            waits = list(si.on_wait)
            keep, hoist = waits[:MAX_WAITS], waits[MAX_WAITS:]
            for i in range(0, len(hoist), MAX_WAITS):
                _ctr[0] += 1
                nop = mybir.InstNoOp(name=f"waitnop-{_ctr[0]}", ins=[], outs=[])
                nop.engine = inst.engine
                nop.sync_info = mybir.SyncInfo(
                    on_wait=hoist[i : i + MAX_WAITS], on_update=[]
                )
                self.nc.register_instruction(nop, overwrite=True)
                _orig_commit(self, nop, lazy_reg_writes=False)
            inst.sync_info = mybir.SyncInfo(on_wait=keep, on_update=list(si.on_update))
        return _orig_commit(self, inst, lazy_reg_writes=lazy_reg_writes)

    tile.TileContext._drain_and_barrier = _drain_and_barrier_split
    tile.TileContext._commit_instruction = _commit_split_waits
    tile.TileContext._wait_split_patched = True

    # use the full usable SBUF on trn2 (default constant is stale)
    import concourse.tile_utils as tile_utils

    tile_utils.max_sbuf_usage = 206 * 1024


def _build_nc():
    import contextlib

    import concourse.bass as bass
    import concourse.tile as tile
    from concourse import mybir

    F32 = mybir.dt.float32
    CDT = mybir.dt.bfloat16 if MM_DTYPE == "bf16" else mybir.dt.float32
    PS = bass.MemorySpace.PSUM
    AF = mybir.ActivationFunctionType

    nc = bass.Bass()
    xT_d = nc.dram_tensor("xT", [E, S], CDT, kind="ExternalInput")
    wqT_d = nc.dram_tensor("wqT", [E, DG], CDT, kind="ExternalInput")
    wkT_d = nc.dram_tensor("wkT", [E, DG], CDT, kind="ExternalInput")
    wvT_d = nc.dram_tensor("wvT", [E, DG], CDT, kind="ExternalInput")
    woT_d = nc.dram_tensor("woT", [DG, E], CDT, kind="ExternalInput")
    sel2_d = nc.dram_tensor("sel2", [2, 128], CDT, kind="ExternalInput")
    out_d = nc.dram_tensor("out", [S, E], CDT, kind="ExternalOutput")

    NE = E // 8 // 128 * 8  # 8 e-tiles
    NE = E // 128
    NT = S // 128  # 16 t/s-tiles
    NNC = S // 512  # 4 s-chunks
    DLY = 2  # av/sums trail scores/exp by DLY steps

    # anti-diagonal (pair, chunk) group order: prereqs drip in gradually
    GROUPS = sorted(
        [(p, n) for p in range(NP) for n in range(NNC)],
        key=lambda pn: (pn[0] + pn[1], pn[1]),
    )

    with tile.TileContext(nc) as tc:
        st = contextlib.ExitStack()
        with st:
            pp = st.enter_context(tc.tile_pool(name="persist", bufs=1))
            stg = st.enter_context(tc.tile_pool(name="stage", bufs=4))
            expp = st.enter_context(tc.tile_pool(name="expp", bufs=6))
            outp = st.enter_context(tc.tile_pool(name="outp", bufs=4))
            zsp = st.enter_context(tc.tile_pool(name="zsp", bufs=2))

            QT = pp.tile([128, NP * S], CDT, tag="QT")  # [128, 8192]
            KT = pp.tile([128, NP * S], CDT, tag="KT")
            Vsb = pp.tile([128, NT * DG], CDT, tag="V")  # [128, 8192]
            onorm = pp.tile([128, NP * S], CDT, tag="onorm")
            woT = pp.tile([128, NP * E], CDT, tag="woT")  # [128, 4096]
            ones = pp.tile([128, 1], CDT, tag="ones")
            nc.gpsimd.memset(ones[:], 1.0)
            # K=2 selector: row 0 -> out partitions 0:64, row 1 -> 64:128
            sel2 = pp.tile([2, 128], CDT, tag="sel2")
            nc.sync.dma_start(sel2[:], sel2_d[:])
            # warmup operand (no DMA dependency)
            warm = pp.tile([128, 256], CDT, tag="warm")
            nc.gpsimd.memset(warm[:], 0.001)
            wscr = pp.tile([128, 16], CDT, tag="wscr")

            xT = pp.tile([128, NE * S], CDT, tag="xT")  # [128, 16384]
            wq = pp.tile([128, NE * DG], CDT, tag="wq")
            wk = pp.tile([128, NE * DG], CDT, tag="wk")
            wv = pp.tile([128, NE * DG], CDT, tag="wv")

            # -------- DMA schedule: x + pair-0 Q/K weight slices first so
            # the upfront projection chains track per-tile arrivals; then
            # V weights (needed early in group 0), remaining pairs, woT.
            for j in range(NE):
                nc.sync.dma_start(
                    xT[:, j * S : (j + 1) * S], xT_d[j * 128 : (j + 1) * 128, :]
                )
                nc.sync.dma_start(
                    wk[:, j * DG : j * DG + 128],
                    wkT_d[j * 128 : (j + 1) * 128, 0:128],
                )
                nc.sync.dma_start(
                    wq[:, j * DG : j * DG + 128],
                    wqT_d[j * 128 : (j + 1) * 128, 0:128],
                )
            for j in range(NE):
                nc.sync.dma_start(
                    wv[:, j * DG : (j + 1) * DG],
                    wvT_d[j * 128 : (j + 1) * 128, :],
                )
            for p in range(1, NP):
                for j in range(NE):
                    nc.sync.dma_start(
                        wk[:, j * DG + p * 128 : j * DG + (p + 1) * 128],
                        wkT_d[j * 128 : (j + 1) * 128, p * 128 : (p + 1) * 128],
                    )
                    nc.sync.dma_start(
                        wq[:, j * DG + p * 128 : j * DG + (p + 1) * 128],
                        wqT_d[j * 128 : (j + 1) * 128, p * 128 : (p + 1) * 128],
                    )
            for k in range(NP):
                nc.sync.dma_start(
                    woT[:, k * E : (k + 1) * E], woT_d[k * 128 : (k + 1) * 128, :]
                )

            # -------- upfront phase: warm the PE p-state + Exp table while
            # x DMA lands, then the minimal projections for group (0,0).
            with tc.tile_pool(name="projps", bufs=3, space=PS) as proj_ps:
                wps = proj_ps.tile([128, 256], F32, tag="pj", name="warmps")
                for r in range(40):
                    nc.tensor.matmul(
                        wps[:],
                        warm[:, 0:128],
                        warm[:],
                        start=(r == 0),
                        stop=(r == 39),
                    )
                # preload the Exp activation table during the DMA wait
                nc.scalar.activation(wscr[:], warm[:, 0:16], AF.Exp, scale=0.125)

                def up_qk(w_sb, dst, p, n):
                    acc = proj_ps.tile([128, 512], F32, tag="pj")
                    for j in range(NE):
                        nc.tensor.matmul(
                            acc[:],
                            w_sb[:, j * DG + p * 128 : j * DG + (p + 1) * 128],
                            xT[:, j * S + n * 512 : j * S + (n + 1) * 512],
                            start=(j == 0),
                            stop=(j == NE - 1),
                        )
                    nc.vector.tensor_copy(
                        dst[:, p * S + n * 512 : p * S + (n + 1) * 512], acc[:]
                    )

                def up_v(i):
                    acc = proj_ps.tile([128, 512], F32, tag="pj")
                    for j in range(NE):
                        nc.tensor.matmul(
                            acc[:],
                            xT[:, j * S + i * 128 : j * S + (i + 1) * 128],
                            wv[:, j * DG : (j + 1) * DG],
                            start=(j == 0),
                            stop=(j == NE - 1),
                        )
                    nc.vector.tensor_copy(Vsb[:, i * DG : (i + 1) * DG], acc[:])

                for n in range(NNC):
                    up_qk(wk, KT, 0, n)
                up_qk(wq, QT, 0, 0)
                for i in range(4):
                    up_v(i)

            sc_ps = st.enter_context(tc.tile_pool(name="scpsum", bufs=2, space=PS))
            av_ps = st.enter_context(tc.tile_pool(name="avpsum", bufs=2, space=PS))
            z_ps = st.enter_context(tc.tile_pool(name="zpsum", bufs=1, space=PS))
            ms_ps = st.enter_context(tc.tile_pool(name="miscpsum", bufs=1, space=PS))

            # -------- drip machinery.
            # qlow: projection + out-projection chains (ms_ps users; chain
            # closures are consecutive, and only qlow touches ms_ps, so the
            # single-buffer ring never interleaves two chains).
            # qhigh: per-pair normalize tasks (rep matmul uses the av_ps
            # ring). Popped before qlow so onorm is emitted ahead of the
            # out-projection chains that read it.
            qhigh = []
            qlow = []
            emitted = set()

            def chain_qk(key, w_sb, dst, p, n):
                cell = {}

                def seg(j0, key=key, p=p, n=n):
                    if j0 == 0:
                        cell["acc"] = ms_ps.tile(
                            [128, 512], F32, tag="ms", name=f"qk{p}_{n}_{key[0]}"
                        )
                    for j in range(j0, j0 + 2):
                        nc.tensor.matmul(
                            cell["acc"][:],
                            w_sb[:, j * DG + p * 128 : j * DG + (p + 1) * 128],
                            xT[:, j * S + n * 512 : j * S + (n + 1) * 512],
                            start=(j == 0),
                            stop=(j == NE - 1),
                        )
                    if j0 + 2 == NE:
                        nc.vector.tensor_copy(
                            dst[:, p * S + n * 512 : p * S + (n + 1) * 512],
                            cell["acc"][:],
                        )
                        emitted.add(key)

                for j0 in range(0, NE, 2):
                    qlow.append((2, lambda j0=j0, s=seg: s(j0)))

            def chain_v(i):
                cell = {}

                def seg(j0, i=i):
                    if j0 == 0:
                        cell["acc"] = ms_ps.tile(
                            [128, 512], F32, tag="ms", name=f"vch{i}"
                        )
                    for j in range(j0, j0 + 2):
                        nc.tensor.matmul(
                            cell["acc"][:],
                            xT[:, j * S + i * 128 : j * S + (i + 1) * 128],
                            wv[:, j * DG : (j + 1) * DG],
                            start=(j == 0),
                            stop=(j == NE - 1),
                        )
                    if j0 + 2 == NE:
                        nc.vector.tensor_copy(
                            Vsb[:, i * DG : (i + 1) * DG], cell["acc"][:]
                        )
                        emitted.add(("v", i))

                for j0 in range(0, NE, 2):
                    qlow.append((2, lambda j0=j0, s=seg: s(j0)))

            def pop_one():
                if qhigh:
                    cost, fn = qhigh.pop(0)
                elif qlow:
                    cost, fn = qlow.pop(0)
                else:
                    return 0
                fn()
                return max(cost, 1)

            def drip(budget):
                while (qhigh or qlow) and budget > 0:
                    budget -= pop_one()

            def ensure(key):
                while key not in emitted:
                    assert qhigh or qlow, f"prereq {key} cannot be satisfied"
                    pop_one()

            # projection drip order = group prereq order (V first: group 0
            # consumes V t-tiles from step ~2 on)
            for i in range(4, NT):
                emitted.discard(("v", i))
                chain_v(i)
            for i in range(4):
                emitted.add(("v", i))
            emitted.add(("q", 0, 0))
            emitted.add(("k", 0))
            seen_q = {(0, 0)}
            seen_k = {0}
            for p, n in GROUPS[1:]:
                if p not in seen_k:
                    seen_k.add(p)
                    for nn in range(NNC):
                        chain_qk(("k", p) if nn == NNC - 1 else ("kpart", p, nn), wk, KT, p, nn)
                if (p, n) not in seen_q:
                    seen_q.add((p, n))
                    chain_qk(("q", p, n), wq, QT, p, n)

            # -------- boundary work for group (p, n): emitted inline (DVE/
            # DMA only, cheap) except the normalize matmul and the chunk's
            # out-projection, which drip.
            uos = {}
            zrecps = {}
            ndone = {n: 0 for n in range(NNC)}

            def boundary(p, n, av, zz):
                uo = stg.tile([128, 512], F32, tag="uo")
                nc.vector.tensor_copy(uo[:], av[:])
                uos[(p, n)] = uo
                zstage = zsp.tile([128, 512], F32, tag="zst")
                nc.vector.tensor_copy(zstage[0:1, :], zz[0:1, :])
                nc.vector.tensor_copy(zstage[32:33, :], zz[32:33, :])
                zpair = zsp.tile([2, 512], F32, tag="zpair")
                nc.sync.dma_start(zpair[0:1, :], zstage[0:1, :])
                nc.sync.dma_start(zpair[1:2, :], zstage[32:33, :])
                zrecf = zsp.tile([2, 512], F32, tag="zrecf")
                with nc.allow_low_precision(reason="softmax 1/Z"):
                    nc.vector.reciprocal(zrecf[:], zpair[:])
                zrecp = zsp.tile([2, 512], CDT, tag="zrecp")
                nc.vector.tensor_copy(zrecp[:], zrecf[:])
                zrecps[(p, n)] = zrecp

                def t_norm(p=p, n=n):
                    zr = zrecps.pop((p, n))
                    rep = av_ps.tile([128, 512], F32, tag="av", name=f"rp{p}{n}")
                    nc.tensor.matmul(rep[:], sel2[:], zr[:])
                    uo_t = uos.pop((p, n))
                    nc.vector.tensor_tensor(
                        onorm[:, p * S + n * 512 : p * S + (n + 1) * 512],
                        uo_t[:],
                        rep[:],
                        mybir.AluOpType.mult,
                    )
                    emitted.add(("norm", p, n))

                qhigh.append((1, t_norm))

                ndone[n] += 1
                if ndone[n] == NP:
                    osbs = {}
                    for i in range(4 * n, 4 * n + 4):
                        def o_pre(i=i):
                            osbs[i] = outp.tile(
                                [128, E], CDT, tag="osb", name=f"osb{i}"
                            )

                        qlow.append((0, o_pre))
                        for eh in (0, 1):
                            cell = {}

                            def o_mm(k0, i=i, eh=eh, cell=cell):
                                if k0 == 0:
                                    cell["ps"] = ms_ps.tile(
                                        [128, 512], F32, tag="ms", name=f"op{i}_{eh}"
                                    )
                                for k in range(k0, k0 + 2):
                                    nc.tensor.matmul(
                                        cell["ps"][:],
                                        onorm[:, k * S + i * 128 : k * S + (i + 1) * 128],
                                        woT[:, k * E + eh * 512 : k * E + (eh + 1) * 512],
                                        start=(k == 0),
                                        stop=(k == NP - 1),
                                    )
                                if k0 + 2 == NP:
                                    nc.vector.tensor_copy(
                                        osbs[i][:, eh * 512 : (eh + 1) * 512],
                                        cell["ps"][:],
                                    )
                                    nc.sync.dma_start(
                                        out_d[
                                            i * 128 : (i + 1) * 128,
                                            eh * 512 : (eh + 1) * 512,
                                        ],
                                        osbs[i][:, eh * 512 : (eh + 1) * 512],
                                    )

                            qlow.append((2, lambda f=o_mm: f(0)))
                            qlow.append((2, lambda f=o_mm: f(2)))

            # -------- attention groups
            def score_step(p, n, t):
                sc = sc_ps.tile([128, 1024], F32, tag="sc")
                nc.tensor.matmul(
                    sc[:, 0:512],
                    KT[0:64, p * S + t * 128 : p * S + (t + 1) * 128],
                    QT[0:64, p * S + n * 512 : p * S + (n + 1) * 512],
                )
                nc.tensor.matmul(
                    sc[:, 512:1024],
                    KT[64:128, p * S + t * 128 : p * S + (t + 1) * 128],
                    QT[64:128, p * S + n * 512 : p * S + (n + 1) * 512],
                )
                ex = expp.tile([128, 1024], CDT, tag="ex")
                nc.scalar.activation(ex[:], sc[:], AF.Exp, scale=0.125)
                return ex

            for p, n in GROUPS:
                ensure(("k", p))
                ensure(("q", p, n))
                av = av_ps.tile([128, 512], F32, tag="av")
                zz = z_ps.tile([128, 512], F32, tag="zz")
                pend = []
                for t in range(NT + DLY):
                    if t < NT:
                        pend.append((t, score_step(p, n, t)))
                    if len(pend) > DLY or t >= NT:
                        pt, ex = pend.pop(0)
                        ensure(("v", pt))
                        voff = pt * DG
                        nc.tensor.matmul(
                            av[0:64, :],
                            Vsb[:, voff + (2 * p) * DK : voff + (2 * p) * DK + DK],
                            ex[:, 0:512],
                            start=(pt == 0),
                            stop=(pt == NT - 1),
                            tile_position=(0, 0),
                            skip_group_check=True,
                        )
                        nc.tensor.matmul(
                            av[64:128, :],
                            Vsb[:, voff + (2 * p + 1) * DK : voff + (2 * p + 1) * DK + DK],
                            ex[:, 512:1024],
                            start=(pt == 0),
                            stop=(pt == NT - 1),
                            tile_position=(0, 64),
                            skip_group_check=True,
                        )
                        nc.tensor.matmul(
                            zz[0:1, :],
                            ones[:, 0:1],
                            ex[:, 0:512],
                            start=(pt == 0),
                            stop=(pt == NT - 1),
                            tile_position=(0, 0),
                            skip_group_check=True,
                        )
                        nc.tensor.matmul(
                            zz[32:33, :],
                            ones[:, 0:1],
                            ex[:, 512:1024],
                            start=(pt == 0),
                            stop=(pt == NT - 1),
                            tile_position=(0, 32),
                            skip_group_check=True,
                        )
                    drip(2)
                boundary(p, n, av, zz)

            while qhigh or qlow:
                pop_one()

    return nc


def kernel(x, Wq, Wk, Wv, Wo):
    global LAST_EXEC_TIME_NS
    _env_setup()
    from concourse.bass_utils import run_bass_kernel_spmd

    x = np.asarray(x, dtype=np.float32)
    Wq = np.asarray(Wq, dtype=np.float32)
    Wk = np.asarray(Wk, dtype=np.float32)
    Wv = np.asarray(Wv, dtype=np.float32)
    Wo = np.asarray(Wo, dtype=np.float32)

    if "nc" not in _cache:
        _cache["nc"] = _build_nc()
    nc = _cache["nc"]

    if MM_DTYPE == "bf16":
        import ml_dtypes

        cdt = ml_dtypes.bfloat16
    else:
        cdt = np.float32

    in_maps = []
    for c in range(NCORES):
        b, hg = c // 2, c % 2
        r = slice(hg * DG, (hg + 1) * DG)
        in_maps.append(
            {
                "xT": np.ascontiguousarray(x[b].T).astype(cdt),
                "wqT": np.ascontiguousarray(Wq[r, :].T).astype(cdt),
                "wkT": np.ascontiguousarray(Wk[r, :].T).astype(cdt),
                "wvT": np.ascontiguousarray(Wv[r, :].T).astype(cdt),
                "woT": np.ascontiguousarray(Wo[:, r].T).astype(cdt),
                "sel2": _SEL2.astype(cdt),
            }
        )

    res = run_bass_kernel_spmd(
        nc, in_maps, core_ids=list(range(NCORES)), trace=TRACE
    )
    if TRACE:
        LAST_EXEC_TIME_NS = res.exec_time_ns

    out = np.empty((B, S, E), dtype=np.float32)
    for b in range(B):
        out[b] = np.asarray(res.results[2 * b]["out"], dtype=np.float32) + np.asarray(
            res.results[2 * b + 1]["out"], dtype=np.float32
        )
    return out
